# revision 1
# baseline (speedup 1.0000x reference)
"""Policy-loss kernel for Trainium2, data-parallel across 8 NeuronCores.

Reference computation (B=16384, m=2048, action has 4*m columns):
    seg_max = max(action.reshape(B, m, 4), axis=-1)        # [B, m]
    a_n     = mean(seg_max, axis=-1)                       # [B]
    v       = log(a_n) * a_n                               # [B]
    loss    = | mean(v * reward) + BETA * mean(v) |        # scalar

Sharding: rows (batch) split evenly over 8 cores (2048 rows each). Each core
streams its 2048x8192 f32 slice through SBUF in 16 tiles of [128, 8192],
computes per-row v with two strided tensor_tensor maxes + one
tensor_tensor_reduce (final max fused with the segment mean) on DVE and the
log on ACT, and returns per-partition partial sums [128, 2] =
(sum v*r, sum v). The host reduces the 8x128x2 partials and applies abs.
"""

import numpy as np

import concourse.bass as bass
import concourse.mybir as mybir
import concourse.tile as tile
from concourse.bass_utils import run_bass_kernel_spmd

BETA = 0.1
N_CORES = 8


def _sem_clear_compat(self, sem):
    """Replacement for BassGpSimd.sem_clear: the EVENT_SEMAPHORE_RANGE_CLEAR
    ISA op (opcode 176) fails this neuronxcc's codegen with "ISA wrong
    length". Emit one EventSemaphore sem-wr-imm 0 per semaphore instead —
    same architectural effect (zero the sems), encodes fine."""
    nums = list(sem) if isinstance(sem, range) else [sem.num]
    inst = None
    for n in nums:
        inst = self.add_instruction(
            mybir.InstEventSemaphore(
                name=f"semclr{n}_{self.bass.next_id()}",
                engine=self.engine,
                ins=[],
                outs=[],
                sync_info=mybir.SyncInfo(
                    on_wait=[],
                    on_update=[
                        mybir.SyncUpdate(
                            sync_type="semaphore",
                            id=n,
                            update_mode="sem-wr-imm",
                            update_value=0,
                        )
                    ],
                ),
            )
        )
    return inst


bass.BassGpSimd.sem_clear = _sem_clear_compat
B = 16384
COLS = 8192          # 4 * mobile_num
M = COLS // 4        # 2048 segments per row
ROWS_PER_CORE = B // N_CORES      # 2048
P = 128                           # SBUF partitions
NT = ROWS_PER_CORE // P           # 16 tiles per core

F32 = mybir.dt.float32


def _build_nc(rows_per_core: int = ROWS_PER_CORE, cols: int = COLS) -> bass.Bass:
    """Raw-bass pipeline (this neuronxcc rejects Tile's multi-wait DMAs):
    SP streams action tiles into a double buffer, DVE does the pairwise max
    tree, ACT does mean+log+v. Manual semaphores, waits are standalone
    sequencer instructions."""
    nt = rows_per_core // P
    m = cols // 4
    Ln = mybir.ActivationFunctionType.Ln
    Copy = mybir.ActivationFunctionType.Copy
    MAX = mybir.AluOpType.max

    nc = bass.Bass()
    a_ext = nc.declare_dram_parameter("action", [rows_per_core, cols], F32, isOutput=False)
    r_ext = nc.declare_dram_parameter("rt", [P, nt], F32, isOutput=False)
    out_ext = nc.declare_dram_parameter("partial", [P, 2], F32, isOutput=True)

    from contextlib import ExitStack

    with ExitStack() as stack:
        at0 = stack.enter_context(nc.sbuf_tensor([P, cols], F32))
        at1 = stack.enter_context(nc.sbuf_tensor([P, cols], F32))
        m1b = stack.enter_context(nc.sbuf_tensor([P, 2 * m], F32))
        seg0 = stack.enter_context(nc.sbuf_tensor([P, m], F32))
        seg1 = stack.enter_context(nc.sbuf_tensor([P, m], F32))
        sg2 = stack.enter_context(nc.sbuf_tensor([P, m], F32))
        a_all = stack.enter_context(nc.sbuf_tensor([P, nt], F32))
        v_all = stack.enter_context(nc.sbuf_tensor([P, nt], F32))
        rt = stack.enter_context(nc.sbuf_tensor([P, nt], F32))
        vr = stack.enter_context(nc.sbuf_tensor([P, nt], F32))
        lg = stack.enter_context(nc.sbuf_tensor([P, 1], F32))
        s1 = stack.enter_context(nc.sbuf_tensor([P, 1], F32))
        s2 = stack.enter_context(nc.sbuf_tensor([P, 1], F32))
        outt = stack.enter_context(nc.sbuf_tensor([P, 2], F32))
        dma_s0 = stack.enter_context(nc.semaphore("dma_s0"))
        dma_s1 = stack.enter_context(nc.semaphore("dma_s1"))
        rt_sem = stack.enter_context(nc.semaphore("rt_sem"))
        out_sem = stack.enter_context(nc.semaphore("out_sem"))
        dve_free = stack.enter_context(nc.semaphore("dve_free"))
        dve_seg = stack.enter_context(nc.semaphore("dve_seg"))
        act_done = stack.enter_context(nc.semaphore("act_done"))
        act_chain = stack.enter_context(nc.semaphore("act_chain"))
        block = stack.enter_context(nc.Block())
        ats = [at0, at1]
        segs = [seg0, seg1]
        dma_s = [dma_s0, dma_s1]

        @block.sync
        def _(sync):
            sync.dma_start(out=rt[:], in_=r_ext[:]).then_inc(rt_sem, 16)
            for i in range(nt):
                if i >= 2:
                    # at[i%2] WAR: max1 of tile i-2 consumed it
                    sync.wait_ge(dve_free, i - 1)
                    # trivially-true direct wait so the slot-sem inc is ordered
                    sync.wait_ge(dma_s[i % 2], 16 * (i // 2))
                sync.dma_start(
                    out=ats[i % 2][:], in_=a_ext[bass.ts(i, P), :]
                ).then_inc(dma_s[i % 2], 16)
            sync.wait_ge(act_done, nt + 2)
            sync.dma_start(out=out_ext[:], in_=outt[:]).then_inc(out_sem, 16)
            sync.wait_ge(out_sem, 16)

        @block.vector
        def _(vector):
            for i in range(nt):
                at = ats[i % 2]
                seg = segs[i % 2]
                vector.wait_ge(dma_s[i % 2], 16 * (i // 2 + 1))
                if i >= 1:
                    # m1b WAR: max2 of tile i-1 read it
                    vector.wait_ge(dve_seg, i)
                vector.tensor_tensor(
                    out=m1b[:], in0=at[:, 0::2], in1=at[:, 1::2], op=MAX
                ).then_inc(dve_free, 1)
                # m1b RAW (same engine, needs explicit sem for ordering model)
                vector.wait_ge(dve_free, i + 1)
                if i >= 2:
                    # seg[i%2] WAR: ACT reduce of tile i-2 read it
                    vector.wait_ge(act_chain, 2 * i - 3)
                vector.tensor_tensor(
                    out=seg[:], in0=m1b[:, 0::2], in1=m1b[:, 1::2], op=MAX
                ).then_inc(dve_seg, 1)
            # final partial sums over the nt per-tile v values
            vector.wait_ge(act_done, nt)
            vector.wait_ge(rt_sem, 16)
            vector.tensor_tensor(
                out=vr[:], in0=v_all[:], in1=rt[:], op=mybir.AluOpType.mult
            ).then_inc(dve_free, 1)
            vector.wait_ge(dve_free, nt + 1)
            vector.reduce_sum(
                out=s1[:], in_=vr[:], axis=mybir.AxisListType.X
            ).then_inc(dve_seg, 1)
            vector.reduce_sum(
                out=s2[:], in_=v_all[:], axis=mybir.AxisListType.X
            ).then_inc(dve_seg, 1)

        @block.scalar
        def _(scalar):
            for i in range(nt):
                seg = segs[i % 2]
                a_n = a_all[:, i : i + 1]
                scalar.wait_ge(dve_seg, i + 1)
                if i >= 1:
                    # sg2 WAW vs reduce of tile i-1
                    scalar.wait_ge(act_chain, 2 * i - 1)
                # out = seg * (1/m); accum_out = mean(seg) = a_n
                scalar.activation(
                    out=sg2[:], in_=seg[:], func=Copy, bias=0.0, scale=1.0 / m,
                    accum_out=a_n,
                ).then_inc(act_chain, 1)
                scalar.wait_ge(act_chain, 2 * i + 1)
                if i >= 1:
                    # lg WAR: v-write of tile i-1 read it
                    scalar.wait_ge(act_done, i)
                scalar.activation(out=lg[:], in_=a_n, func=Ln).then_inc(act_chain, 1)
                scalar.wait_ge(act_chain, 2 * i + 2)
                # v = log(a_n) * a_n into column i of v_all
                scalar.activation(
                    out=v_all[:, i : i + 1], in_=lg[:], func=Copy, bias=0.0,
                    scale=a_n,
                ).then_inc(act_done, 1)
            scalar.wait_ge(dve_seg, nt + 1)
            scalar.copy(out=outt[:, 0:1], in_=s1[:]).then_inc(act_done, 1)
            scalar.wait_ge(dve_seg, nt + 2)
            scalar.copy(out=outt[:, 1:2], in_=s2[:]).then_inc(act_done, 1)

    return nc


def _make_in_maps(reward: np.ndarray, action: np.ndarray, n_cores: int = N_CORES):
    rows_per_core = action.shape[0] // n_cores
    nt = rows_per_core // P
    a_sh = np.ascontiguousarray(action, dtype=np.float32).reshape(
        n_cores, rows_per_core, action.shape[1]
    )
    # rt[c][p, i] = reward[c*rows_per_core + i*P + p]
    r_sh = np.ascontiguousarray(reward, dtype=np.float32).reshape(
        n_cores, nt, P
    ).transpose(0, 2, 1)
    return [
        {"action": a_sh[c], "rt": np.ascontiguousarray(r_sh[c])}
        for c in range(n_cores)
    ]


def _run(q_eval, reward, action, trace: bool = False):
    nc = _build_nc()
    in_maps = _make_in_maps(np.asarray(reward), np.asarray(action))
    res = run_bass_kernel_spmd(nc, in_maps, list(range(N_CORES)), trace=trace)
    partials = np.stack([res.results[c]["partial"] for c in range(N_CORES)])
    s1 = float(partials[:, :, 0].sum(dtype=np.float64))
    s2 = float(partials[:, :, 1].sum(dtype=np.float64))
    loss = np.float32(abs(np.float32(s1 / B) + np.float32(BETA) * np.float32(s2 / B)))
    return np.asarray(loss, dtype=np.float32), res


def kernel(q_eval, reward, action):
    out, _ = _run(q_eval, reward, action)
    return out



# revision 9
# speedup vs baseline: 1.0026x; 1.0026x over previous
"""Policy-loss kernel for Trainium2, data-parallel across 8 NeuronCores.

Reference computation (B=16384, m=2048, action has 4*m columns):
    seg_max = max(action.reshape(B, m, 4), axis=-1)        # [B, m]
    a_n     = mean(seg_max, axis=-1)                       # [B]
    v       = log(a_n) * a_n                               # [B]
    loss    = | mean(v * reward) + BETA * mean(v) |        # scalar

Sharding: rows (batch) split evenly over 8 cores (2048 rows each). Each core
streams its 2048x8192 f32 slice through SBUF on the SP HWDGE ring: 15 full
[128, 8192] row-tiles double-buffered, then the last row-tile split into 9
column chunks (7x1024 + 2x512 cols) so the final max-tree pipelines inside
the DMA stream instead of serializing after it. DVE does a pairwise max then
a fused scalar_tensor_tensor (final max + unscaled segment sum in one pass);
ACT only loads rt (on its own HWDGE ring, keeping SP free for the action
stream) and does one batched Ln(ssum/m) at the end. Each core returns
[128, 2] = m*(sum v*r, sum v) partials; the host reduces the 8x128x2
partials, divides by m, and applies abs.
"""

import numpy as np

import concourse.bass as bass
import concourse.mybir as mybir
import concourse.tile as tile
from concourse.bass_utils import run_bass_kernel_spmd

BETA = 0.1
N_CORES = 8


def _sem_clear_compat(self, sem):
    """Replacement for BassGpSimd.sem_clear: the EVENT_SEMAPHORE_RANGE_CLEAR
    ISA op (opcode 176) fails this neuronxcc's codegen with "ISA wrong
    length". Emit one EventSemaphore sem-wr-imm 0 per semaphore instead —
    same architectural effect (zero the sems), encodes fine."""
    nums = list(sem) if isinstance(sem, range) else [sem.num]
    inst = None
    for n in nums:
        inst = self.add_instruction(
            mybir.InstEventSemaphore(
                name=f"semclr{n}_{self.bass.next_id()}",
                engine=self.engine,
                ins=[],
                outs=[],
                sync_info=mybir.SyncInfo(
                    on_wait=[],
                    on_update=[
                        mybir.SyncUpdate(
                            sync_type="semaphore",
                            id=n,
                            update_mode="sem-wr-imm",
                            update_value=0,
                        )
                    ],
                ),
            )
        )
    return inst


bass.BassGpSimd.sem_clear = _sem_clear_compat
B = 16384
COLS = 8192          # 4 * mobile_num
M = COLS // 4        # 2048 segments per row
ROWS_PER_CORE = B // N_CORES      # 2048
P = 128                           # SBUF partitions
NT = ROWS_PER_CORE // P           # 16 row-tiles per core
NBIG = NT - 1                     # full-width row-tiles
CHUNK_COLS = [1024] * 7 + [512] * 2   # column split of the last row-tile
NCH = len(CHUNK_COLS)
NUNIT = NBIG + NCH                # max-tree units (one TTR each)

F32 = mybir.dt.float32


def _build_nc(rows_per_core: int = ROWS_PER_CORE, cols: int = COLS) -> bass.Bass:
    """Raw-bass pipeline (this neuronxcc rejects Tile's multi-wait DMAs and
    the custom-ISA TENSOR_TENSOR_REDUCE — "ISA wrong length"): SP streams
    action tiles into a double buffer, DVE does max1 + fused max2/segment-sum
    (scalar_tensor_tensor, a standard BIR op), ACT does rt load + one Ln.
    Manual semaphores, waits are standalone sequencer instructions."""
    m = cols // 4
    Ln = mybir.ActivationFunctionType.Ln
    MAX = mybir.AluOpType.max
    MULT = mybir.AluOpType.mult
    BYP = mybir.AluOpType.bypass

    nc = bass.Bass()
    a_ext = nc.declare_dram_parameter("action", [rows_per_core, cols], F32, isOutput=False)
    r_ext = nc.declare_dram_parameter("rt", [P, NT], F32, isOutput=False)
    out_ext = nc.declare_dram_parameter("partial", [P, 2], F32, isOutput=True)

    from contextlib import ExitStack

    with ExitStack() as stack:
        at0 = stack.enter_context(nc.sbuf_tensor([P, cols], F32))
        at1 = stack.enter_context(nc.sbuf_tensor([P, cols], F32))
        m1b = stack.enter_context(nc.sbuf_tensor([P, cols // 2], F32))
        sg2 = stack.enter_context(nc.sbuf_tensor([P, cols // 4], F32))
        ssum = stack.enter_context(nc.sbuf_tensor([P, NT], F32))
        csum = stack.enter_context(nc.sbuf_tensor([P, NCH], F32))
        lg = stack.enter_context(nc.sbuf_tensor([P, NT], F32))
        lgd = stack.enter_context(nc.sbuf_tensor([P, 1], F32))
        vp = stack.enter_context(nc.sbuf_tensor([P, NT], F32))
        vpr = stack.enter_context(nc.sbuf_tensor([P, NT], F32))
        rt = stack.enter_context(nc.sbuf_tensor([P, NT], F32))
        outt = stack.enter_context(nc.sbuf_tensor([P, 2], F32))
        dma_s0 = stack.enter_context(nc.semaphore("dma_s0"))
        dma_s1 = stack.enter_context(nc.semaphore("dma_s1"))
        rt_sem = stack.enter_context(nc.semaphore("rt_sem"))
        out_sem = stack.enter_context(nc.semaphore("out_sem"))
        dve_free = stack.enter_context(nc.semaphore("dve_free"))
        dve_seg = stack.enter_context(nc.semaphore("dve_seg"))
        act_sem = stack.enter_context(nc.semaphore("act_sem"))
        block = stack.enter_context(nc.Block())
        ats = [at0, at1]
        dma_s = [dma_s0, dma_s1]
        n_buf1_big = NBIG // 2  # big tiles landing in at1 (u odd): 7

        @block.sync
        def _(sync):
            for u in range(NBIG):
                if u >= 2:
                    # at[u%2] WAR: max1 of tile u-2 consumed it
                    sync.wait_ge(dve_free, u - 1)
                    # trivially-true direct wait so the slot-sem inc is ordered
                    sync.wait_ge(dma_s[u % 2], 16 * (u // 2))
                sync.dma_start(
                    out=ats[u % 2][:], in_=a_ext[bass.ts(u, P), :]
                ).then_inc(dma_s[u % 2], 16)
            # last row-tile, column-chunked into at1
            off = 0
            for c, w in enumerate(CHUNK_COLS):
                if c == 0:
                    # at1 WAR: max1 of tile NBIG-2 (last at1 user) consumed it
                    sync.wait_ge(dve_free, NBIG - 1)
                    sync.wait_ge(dma_s1, 16 * n_buf1_big)
                sync.dma_start(
                    out=at1[:, off : off + w],
                    in_=a_ext[bass.ts(NBIG, P), off : off + w],
                ).then_inc(dma_s1, 16)
                off += w
            sync.wait_ge(dve_seg, NUNIT + 3)
            sync.dma_start(out=out_ext[:], in_=outt[:]).then_inc(out_sem, 16)
            sync.wait_ge(out_sem, 16)

        @block.vector
        def _(vector):
            for u in range(NBIG):
                at = ats[u % 2]
                vector.wait_ge(dma_s[u % 2], 16 * (u // 2 + 1))
                if u >= 1:
                    # m1b WAR ordering token: TTR of unit u-1 read it
                    vector.wait_ge(dve_seg, u)
                vector.tensor_tensor(
                    out=m1b[:], in0=at[:, 0::2], in1=at[:, 1::2], op=MAX
                ).then_inc(dve_free, 1)
                # m1b RAW (same engine, needs explicit sem for ordering model)
                vector.wait_ge(dve_free, u + 1)
                # seg = max(m1b even, m1b odd); ssum[:,u] = sum(seg)
                vector.scalar_tensor_tensor(
                    out=sg2[:], in0=m1b[:, 0::2], scalar=0.0,
                    in1=m1b[:, 1::2], op0=BYP, op1=MAX,
                    accum_out=ssum[:, u : u + 1],
                ).then_inc(dve_seg, 1)
            off = 0
            for c, w in enumerate(CHUNK_COLS):
                k = NBIG + c
                vector.wait_ge(dma_s1, 16 * n_buf1_big + 16 * (c + 1))
                # m1b WAR ordering token: STT of unit k-1 read it
                vector.wait_ge(dve_seg, k)
                vector.tensor_tensor(
                    out=m1b[:, 0 : w // 2],
                    in0=at1[:, off : off + w : 2],
                    in1=at1[:, off + 1 : off + w : 2],
                    op=MAX,
                ).then_inc(dve_free, 1)
                vector.wait_ge(dve_free, k + 1)
                vector.scalar_tensor_tensor(
                    out=sg2[:, 0 : w // 4],
                    in0=m1b[:, 0 : w // 2 : 2], scalar=0.0,
                    in1=m1b[:, 1 : w // 2 : 2], op0=BYP, op1=MAX,
                    accum_out=csum[:, c : c + 1],
                ).then_inc(dve_seg, 1)
                off += w
            # combine chunk sums into the last row-tile's segment sum
            vector.reduce_sum(
                out=ssum[:, NBIG : NBIG + 1], in_=csum[:], axis=mybir.AxisListType.X
            ).then_inc(dve_seg, 1)
            # end chain: vp = ln(ssum/m) * ssum (= m*v); partials carry the
            # extra factor m, divided out on the host
            vector.wait_ge(act_sem, 2)
            vector.tensor_tensor(
                out=vp[:], in0=lg[:], in1=ssum[:], op=MULT
            ).then_inc(dve_free, 1)
            vector.wait_ge(dve_free, NUNIT + 1)
            vector.wait_ge(rt_sem, 16)
            vector.tensor_tensor(
                out=vpr[:], in0=vp[:], in1=rt[:], op=MULT
            ).then_inc(dve_free, 1)
            vector.wait_ge(dve_free, NUNIT + 2)
            vector.reduce_sum(
                out=outt[:, 0:1], in_=vpr[:], axis=mybir.AxisListType.X
            ).then_inc(dve_seg, 1)
            vector.reduce_sum(
                out=outt[:, 1:2], in_=vp[:], axis=mybir.AxisListType.X
            ).then_inc(dve_seg, 1)

        @block.scalar
        def _(scalar):
            # rt load on the ACT HWDGE ring keeps SP's ring free for the
            # action stream
            scalar.dma_start(out=rt[:], in_=r_ext[:]).then_inc(rt_sem, 16)
            scalar.wait_ge(rt_sem, 16)
            # Ln(0*rt + 1) = 0: pulls the ACT table load into the stream
            scalar.activation(
                out=lgd[:], in_=rt[:, 0:1], func=Ln, bias=1.0, scale=0.0
            ).then_inc(act_sem, 1)
            scalar.wait_ge(dve_seg, NUNIT + 1)
            # lg = Ln(ssum * 1/m) = Ln(a_n)
            scalar.activation(
                out=lg[:], in_=ssum[:], func=Ln, scale=1.0 / m
            ).then_inc(act_sem, 1)

    return nc


def _make_in_maps(reward: np.ndarray, action: np.ndarray, n_cores: int = N_CORES):
    rows_per_core = action.shape[0] // n_cores
    nt = rows_per_core // P
    a_sh = np.ascontiguousarray(action, dtype=np.float32).reshape(
        n_cores, rows_per_core, action.shape[1]
    )
    # rt[c][p, i] = reward[c*rows_per_core + i*P + p]
    r_sh = np.ascontiguousarray(reward, dtype=np.float32).reshape(
        n_cores, nt, P
    ).transpose(0, 2, 1)
    return [
        {"action": a_sh[c], "rt": np.ascontiguousarray(r_sh[c])}
        for c in range(n_cores)
    ]


def _run(q_eval, reward, action, trace: bool = False):
    nc = _build_nc()
    in_maps = _make_in_maps(np.asarray(reward), np.asarray(action))
    res = run_bass_kernel_spmd(nc, in_maps, list(range(N_CORES)), trace=trace)
    partials = np.stack([res.results[c]["partial"] for c in range(N_CORES)])
    # device partials carry an extra factor of M (sums of m*v)
    s1 = float(partials[:, :, 0].sum(dtype=np.float64)) / M
    s2 = float(partials[:, :, 1].sum(dtype=np.float64)) / M
    loss = np.float32(abs(np.float32(s1 / B) + np.float32(BETA) * np.float32(s2 / B)))
    return np.asarray(loss, dtype=np.float32), res


def kernel(q_eval, reward, action):
    out, _ = _run(q_eval, reward, action)
    return out
